# revision 2
# baseline (speedup 1.0000x reference)
"""Trainium2 Bass kernel v3: sliding-window causal MHA with RoPE + ALiBi.

Sharding: 8 cores = 4 batches x 2 head-groups (8 heads each).

v3 = v2 structure + hi/lo-split fp8e4 DoubleRow projections:
  x @ w ~= xh@wh + xl@wh + xh@wl   (xh=fp8(x), xl=fp8(x-xh); w pre-scaled by
  32 host-side so wl avoids the fp8 subnormal floor; evac rescales by 1/32).
  Each term is a DoubleRow matmul contracting a PAIR of 128-c k-tiles
  (lhsT [128,2,M], rhs [128,2,N], both fp8) -> 24 matmuls per 512-wide psum
  instead of 16 bf16 matmuls, at 0.5 cyc/row: 0.75x cycles of bf16 overall.
  Validated on CPU: rel_err 3.0e-3 (better than all-bf16 3.39e-3).
Attention (scores, PV, ones-sums) stays bf16.  v-bias folded into host bo.
"""
import sys
sys.path.insert(0, '/opt/trn_rl_repo')
from contextlib import ExitStack

import numpy as np
import ml_dtypes
import concourse.bass as bass
import concourse.bacc as bacc
import concourse.mybir as mybir
import concourse.tile as tile

L, N, C, H, D, W = 1024, 4, 2048, 16, 128, 512
HPC = 8                       # heads per core
GD = HPC * D                  # 1024 head-dims per core
SCALE = 1.0 / float(np.sqrt(D))
WSCALE = 32.0                 # host pre-scale on weights (exact power of 2)
F32 = mybir.dt.float32
BF16 = mybir.dt.bfloat16
F8 = mybir.dt.float8e4
MMDT = BF16
DR = mybir.MatmulPerfMode.DoubleRow
AF = mybir.ActivationFunctionType
NT_C = C // 128               # 16 contraction tiles over embed dim
NT_HD = GD // 128             # 8 head tiles (1 head each, D=128)
NT_T = L // 128               # 8 token tiles
QG = 256                      # query-group width
NQG = L // QG                 # 4
MASK_W = 1408                 # master mask width: covers rel = dj - y + MASK_C0
MASK_C0 = 384
N_PREWARM = 60                # dummy matmuls to ramp PE clock during first DMAs


def jtiles(i0):
    return list(range(max(0, i0 - W), min(i0 + QG, L) - 128 + 1, 128))


def emit(tc, t):
    nc = tc.nc
    from concourse.alu_op_type import AluOpType

    # ---------------- prewarm + consts ----------------
    cpool = tc.alloc_tile_pool(name="const", bufs=1, side="left")
    warm = cpool.tile([128, 128], MMDT, tag="warm")
    nc.vector.memset(warm[:], 0.0)
    ones = cpool.tile([128, 128], MMDT, tag="ones")
    nc.sync.dma_start(ones[:], t["ones"][:])
    cos2 = cpool.tile([128, L], MMDT, tag="cos2")
    sin2 = cpool.tile([128, L], MMDT, tag="sin2")
    bq_s = cpool.tile([128, NT_HD], F32, tag="bq")
    bk_s = cpool.tile([128, NT_HD], F32, tag="bk")

    # long-lived left-stack tiles
    vp = tc.alloc_tile_pool(name="vp", bufs=1, side="left")
    vts = [vp.tile([128, GD], MMDT, tag=f"v{tt}", name=f"v{tt}") for tt in range(NT_T)]
    qkp = tc.alloc_tile_pool(name="qkp", bufs=1, side="left")
    qts = [qkp.tile([128, L], MMDT, tag=f"q{m}", name=f"q{m}") for m in range(NT_HD)]
    kts = [qkp.tile([128, L], MMDT, tag=f"k{m}", name=f"k{m}") for m in range(NT_HD)]
    # attention output, fp8 hi/lo packed [128, htile, 2, L]
    apool = tc.alloc_tile_pool(name="apool", bufs=1, side="left")
    aa = apool.tile([128, NT_HD, 2, L], F8, tag="aa")

    pool1 = tc.alloc_tile_pool(name="pool1", bufs=8, space="PSUM")

    def psum():
        return pool1.tile([128, 512], F32, tag="pp", name="pp")

    # prewarm: dummy matmuls on uninitialized tile (result never read)
    wps = psum()
    for i in range(N_PREWARM):
        nc.tensor.matmul(wps[:, 0:64], warm[:], warm[:, 0:64],
                         start=True, stop=True)

    # ---------------- phase A: v-proj ----------------
    xp = tc.alloc_tile_pool(name="xp", bufs=1, side="right")
    xx = xp.tile([128, NT_C, 2, L], F8, tag="xx")     # x hi/lo packed
    ws = tc.alloc_tile_pool(name="ws", bufs=4, side="right")
    rp = tc.alloc_tile_pool(name="rp", bufs=4, side="right")
    wvp = tc.alloc_tile_pool(name="wvp", bufs=1, side="right")
    wvv = wvp.tile([128, NT_C, 2, GD], F8, tag="wvv")  # wv*32 hi/lo packed
    for n in range(NT_C):
        (nc.sync if n % 2 == 0 else nc.scalar).dma_start(
            xx[:, n, :, :], t["x8"][n])
        (nc.scalar if n % 2 == 0 else nc.sync).dma_start(
            wvv[:, n, :, :], t["wv8"][n])
    nc.sync.dma_start(cos2[:], t["cos2"][:])
    nc.scalar.dma_start(sin2[:], t["sin2"][:])
    nc.sync.dma_start(bq_s[:], t["bq"][:])
    nc.scalar.dma_start(bk_s[:], t["bk"][:])

    def hilo_mms(ps, lhs_t, rhs_t, kp, nkp, lslice, rslice):
        """3-term hi/lo DoubleRow matmuls for k-pair kp into psum ps.
        lhs_t/rhs_t are 4D packed tiles [128, nk, 2, M]."""
        first = (kp == 0)
        last = (kp == nkp - 1)
        terms = ((0, 0), (1, 0), (0, 1))   # (lhs hi/lo, rhs hi/lo)
        for ti, (la, rb) in enumerate(terms):
            nc.tensor.matmul(
                ps,
                lhs_t[:, 2 * kp:2 * kp + 2, la, lslice],
                rhs_t[:, 2 * kp:2 * kp + 2, rb, rslice],
                start=(first and ti == 0), stop=(last and ti == 2),
                perf_mode=DR)

    # wave 1: k-pair-outer over 8 banks (DMA-paced)
    pss = [psum() for _ in range(8)]
    for kp in range(8):
        for k in range(8):
            tt, i2 = k // 2, k % 2
            hilo_mms(pss[k][:], xx, wvv, kp, 8,
                     slice(tt * 128, (tt + 1) * 128),
                     slice(i2 * 512, (i2 + 1) * 512))
    for k in range(8):
        tt, i2 = k // 2, k % 2
        nc.scalar.activation(
            vts[tt][:, i2 * 512:(i2 + 1) * 512], pss[k][:],
            AF.Identity, scale=1.0 / WSCALE)
    # wave 2: kp-inner per bank (staggered completion)
    for k in range(8):
        tt, i2 = 4 + k // 2, k % 2
        ps = psum()
        for kp in range(8):
            hilo_mms(ps[:], xx, wvv, kp, 8,
                     slice(tt * 128, (tt + 1) * 128),
                     slice(i2 * 512, (i2 + 1) * 512))
        nc.scalar.activation(
            vts[tt][:, i2 * 512:(i2 + 1) * 512], ps[:],
            AF.Identity, scale=1.0 / WSCALE)

    wvp.release()

    # prefetch expb/wo into wvp's released space during phase B
    mpool = tc.alloc_tile_pool(name="mp", bufs=1, side="right")
    expbs = []
    for h in range(HPC):
        eb = mpool.tile([128, MASK_W], MMDT, tag=f"expb{h}", name=f"expb{h}")
        (nc.sync if h % 2 == 0 else nc.scalar).dma_start(eb[:], t["expb"][h])
        expbs.append(eb)
    wop = tc.alloc_tile_pool(name="wop", bufs=1, side="right")
    wov = wop.tile([128, NT_HD, 2, C], F8, tag="wov")  # wo*32 hi/lo packed
    for hh in range(NT_HD):
        (nc.scalar if hh % 2 == 0 else nc.sync).dma_start(
            wov[:, hh, :, :], t["wo8"][hh])

    # ---------------- phase B: q/k-proj + rope ----------------
    dq = 0
    for m in range(NT_HD):
        for wname, dst, bias_s in (("wq8", qts, bq_s), ("wk8", kts, bk_s)):
            wt = ws.tile([128, NT_C, 2, 128], F8, tag="wqk", name=f"w{wname}{m}")
            (nc.sync if dq % 2 == 0 else nc.scalar).dma_start(
                wt[:], t[wname][m])
            dq += 1
            for i2 in range(2):
                ps = psum()
                for kp in range(8):
                    # lhsT = weight tile [c,2,128 hd], rhs = x [c,2,512 t]
                    hilo_mms(ps[:], wt, xx, kp, 8,
                             slice(0, 128),
                             slice(i2 * 512, (i2 + 1) * 512))
                sl = dst[m][:, i2 * 512:(i2 + 1) * 512]
                csl = slice(i2 * 512, (i2 + 1) * 512)
                qw = rp.tile([128, 512], MMDT, tag="qw", name="qw")
                nc.scalar.activation(
                    qw[:], ps[:],
                    AF.Identity, bias=bias_s[:, m:m + 1], scale=1.0 / WSCALE)
                rot = rp.tile([128, 512], MMDT, tag="rot", name="rot")
                nc.vector.tensor_copy(rot[0:64, :], qw[64:128, :])
                nc.vector.tensor_copy(rot[64:128, :], qw[0:64, :])
                nc.vector.tensor_mul(qw[:], qw[:], cos2[:, csl])
                nc.vector.tensor_mul(rot[:], rot[:], sin2[:, csl])
                nc.vector.tensor_add(sl, qw[:], rot[:])

    pool1.release()

    # ---------------- phase C+D fused ----------------
    with tc.tile_pool(name="spp", bufs=4, space="PSUM") as spp, \
         tc.tile_pool(name="mpp", bufs=2, space="PSUM") as mpp, \
         tc.tile_pool(name="cw", bufs=6, side="right") as cw, \
         tc.tile_pool(name="og", bufs=3, side="right") as og:
        for gi in range(NQG):
            i0 = gi * QG
            js = jtiles(i0)
            pairs = [(js[2 * p], js[2 * p + 1]) for p in range(len(js) // 2)]
            for h in range(HPC):
                ab = mpp.tile([128, 512], F32, tag="ab", name="ab", bufs=2)
                pts = []
                for pi, (j0, j1) in enumerate(pairs):
                    sb = spp.tile([128, 512], F32, tag="s", name="s_ps")
                    for q2, jj in ((0, j0), (1, j1)):
                        nc.tensor.matmul(
                            sb[:, q2 * QG:(q2 + 1) * QG],
                            kts[h][:, jj:jj + 128],
                            qts[h][:, i0:i0 + QG],
                            start=True, stop=True)
                    e = cw.tile([128, 512], MMDT, tag="e", name="e")
                    nc.scalar.activation(e[:], sb[:], AF.Exp, scale=SCALE)
                    for q2, jj in ((0, j0), (1, j1)):
                        pT = cw.tile([128, QG], MMDT, tag="pT", name="pT")
                        soff = MASK_C0 - (jj - i0)
                        eng = nc.vector if (pi + q2) % 2 == 0 else nc.gpsimd
                        eng.tensor_mul(pT[:], e[:, q2 * QG:(q2 + 1) * QG],
                                       expbs[h][:, soff:soff + QG])
                        pts.append((jj, pT))
                # start=True zeroes the WHOLE psum bank (hw-verified), so only
                # the very first matmul starts; everything else accumulates.
                for idx, (jj, pT) in enumerate(pts):
                    sp_ = (idx == len(pts) - 1)
                    nc.tensor.matmul(
                        ab[:, 0:QG],
                        vts[jj // 128][:, h * 128:(h + 1) * 128],
                        pT[:], start=(idx == 0), stop=sp_, skip_group_check=True)
                    nc.tensor.matmul(
                        ab[:, QG:2 * QG],
                        ones[:],
                        pT[:], start=False, stop=sp_, skip_group_check=True)
                rec = cw.tile([128, QG], F32, tag="rec", name="rec", bufs=4)
                nc.vector.reciprocal(rec[:], ab[:, QG:2 * QG])
                aw = cw.tile([128, QG], F32, tag="aw", name="aw", bufs=4)
                nc.vector.scalar_tensor_tensor(
                    aw[:], ab[:, 0:QG], 1.0, rec[:],
                    op0=AluOpType.mult, op1=AluOpType.mult)
                # hi/lo fp8 split of attn row block
                asl = slice(i0, i0 + QG)
                nc.scalar.activation(aa[:, h, 0, asl], aw[:],
                                     AF.Identity, scale=1.0)
                nc.vector.tensor_sub(aa[:, h, 1, asl], aw[:], aa[:, h, 0, asl])
            # out-proj for this group's 2 token tiles
            for tt in (2 * gi, 2 * gi + 1):
                tsl = slice(tt * 128, (tt + 1) * 128)
                for cc in range(4):
                    ps = mpp.tile([128, 512], F32, tag="po", name="psD", bufs=2)
                    for kp in range(NT_HD // 2):
                        hilo_mms(ps[:], aa, wov, kp, NT_HD // 2,
                                 tsl, slice(cc * 512, (cc + 1) * 512))
                    o = og.tile([128, 512], F32, tag="o", name="o")
                    nc.scalar.activation(o[:], ps[:], AF.Identity,
                                         scale=1.0 / WSCALE)
                    ((nc.sync if cc % 2 == 0 else nc.scalar)).dma_start(
                        t["out"][tsl, cc * 512:(cc + 1) * 512], o[:])

    wop.release()
    mpool.release()
    rp.release()
    ws.release()
    xp.release()
    apool.release()
    qkp.release()
    vp.release()
    cpool.release()


def build_nc(enable_asserts=False, reps=1):
    nc = bacc.Bacc("TRN2", target_bir_lowering=False, debug=False,
                   enable_asserts=enable_asserts, num_devices=8)
    t = {}
    t["x8"] = nc.dram_tensor("x8", [NT_C, 128, 2, L], F8, kind="ExternalInput").ap()
    t["wv8"] = nc.dram_tensor("wv8", [NT_C, 128, 2, GD], F8, kind="ExternalInput").ap()
    t["wq8"] = nc.dram_tensor("wq8", [NT_HD, 128, NT_C, 2, 128], F8, kind="ExternalInput").ap()
    t["wk8"] = nc.dram_tensor("wk8", [NT_HD, 128, NT_C, 2, 128], F8, kind="ExternalInput").ap()
    t["wo8"] = nc.dram_tensor("wo8", [NT_HD, 128, 2, C], F8, kind="ExternalInput").ap()
    t["cos2"] = nc.dram_tensor("cos2", [128, L], MMDT, kind="ExternalInput").ap()
    t["sin2"] = nc.dram_tensor("sin2", [128, L], MMDT, kind="ExternalInput").ap()
    t["bq"] = nc.dram_tensor("bq", [128, NT_HD], F32, kind="ExternalInput").ap()
    t["bk"] = nc.dram_tensor("bk", [128, NT_HD], F32, kind="ExternalInput").ap()
    t["expb"] = nc.dram_tensor("expb", [HPC, 128, MASK_W], MMDT, kind="ExternalInput").ap()
    t["ones"] = nc.dram_tensor("ones", [128, 128], MMDT, kind="ExternalInput").ap()
    t["out"] = nc.dram_tensor("out", [L, C], F32, kind="ExternalOutput").ap()
    with tile.TileContext(nc) as tc:
        for _ in range(reps):
            emit(tc, t)
    nc.compile()
    return nc


F8NP = ml_dtypes.float8_e4m3fn


def split8(a):
    hi = np.asarray(a, np.float32).astype(F8NP)
    lo = (np.asarray(a, np.float32) - hi.astype(np.float32)).astype(F8NP)
    return hi, lo


def marshal(inputs):
    x = np.asarray(inputs["x"], np.float32)
    wq = np.asarray(inputs["wq"], np.float32)
    wkv = np.asarray(inputs["wkv"], np.float32)
    wo = np.asarray(inputs["wo"], np.float32)
    bq = np.asarray(inputs["bq"], np.float32)
    bkv = np.asarray(inputs["bkv"], np.float32)
    alibi = np.asarray(inputs["alibi_slopes"], np.float32)
    wk_full, wv_full = wkv[:C], wkv[C:]
    bk_full = bkv[:C]

    perm = np.concatenate([np.arange(0, D, 2), np.arange(1, D, 2)])
    head_perm = np.concatenate([h * D + perm for h in range(H)])
    wq_p, wk_p = wq[head_perm], wk_full[head_perm]
    bq_p, bk_p = bq[head_perm], bk_full[head_perm]

    t_abs = np.arange(W, W + L, dtype=np.float64)
    inv = 1.0 / (10000.0 ** (np.arange(0, D, 2, dtype=np.float64) / D))
    fr = np.outer(t_abs, inv)
    cosT = np.cos(fr).T.astype(np.float32)
    sinT = np.sin(fr).T.astype(np.float32)
    cos2 = np.ascontiguousarray(np.concatenate([cosT, cosT], 0))
    sin2 = np.ascontiguousarray(np.concatenate([-sinT, sinT], 0))

    dj = np.arange(128)[:, None]
    y = np.arange(MASK_W)[None, :]
    rel = (dj - y + MASK_C0).astype(np.float64)
    win = (rel <= 0) & (rel >= -W)

    bf = ml_dtypes.bfloat16
    in_maps = []
    for core in range(8):
        b, g = divmod(core, 2)
        gs = slice(g * GD, (g + 1) * GD)
        xb = x[:, b, :]
        # x8: [NT_C, 128, 2, L] hi/lo packed
        xT_m = np.ascontiguousarray(xb.T).reshape(NT_C, 128, L)
        xh, xl = split8(xT_m)
        x8 = np.ascontiguousarray(np.stack([xh, xl], axis=2))
        # wv8: [NT_C, 128, 2, GD], scaled by 32
        wv_m = np.ascontiguousarray(wv_full[gs].T).reshape(NT_C, 128, GD)
        wvh, wvl = split8(wv_m * WSCALE)
        wv8 = np.ascontiguousarray(np.stack([wvh, wvl], axis=2))
        # wq8/wk8: [NT_HD, 128, NT_C, 2, 128], scaled by 32
        wq_m = wq_p[gs].reshape(NT_HD, 128, NT_C, 128).transpose(0, 3, 2, 1)
        wk_m = wk_p[gs].reshape(NT_HD, 128, NT_C, 128).transpose(0, 3, 2, 1)
        wqh, wql = split8(wq_m * WSCALE)
        wkh, wkl = split8(wk_m * WSCALE)
        wq8 = np.ascontiguousarray(np.stack([wqh, wql], axis=3))
        wk8 = np.ascontiguousarray(np.stack([wkh, wkl], axis=3))
        # wo8: [NT_HD, 128, 2, C] (hd-inner on partition), scaled by 32
        wo_m = np.ascontiguousarray(wo[:, gs].T).reshape(NT_HD, 128, C)
        woh, wol = split8(wo_m * WSCALE)
        wo8 = np.ascontiguousarray(np.stack([woh, wol], axis=2))
        bq_m = np.ascontiguousarray(bq_p[gs].reshape(NT_HD, 128).T)
        bk_m = np.ascontiguousarray(bk_p[gs].reshape(NT_HD, 128).T)
        expb = np.zeros((HPC, 128, MASK_W), np.float32)
        for hh in range(HPC):
            s = float(alibi[g * HPC + hh])
            expb[hh] = np.where(win, np.exp(s * rel), 0.0).astype(np.float32)
        in_maps.append(dict(
            x8=x8, wv8=wv8, wq8=wq8, wk8=wk8, wo8=wo8,
            cos2=cos2.astype(bf), sin2=sin2.astype(bf),
            bq=bq_m, bk=bk_m, expb=expb.astype(bf),
            ones=np.ones((128, 128), bf)))
    return in_maps


def gather(results, inputs):
    wo = np.asarray(inputs["wo"], np.float32)
    bo = np.asarray(inputs["bo"], np.float32)
    bv_full = np.asarray(inputs["bkv"], np.float32)[C:]
    bo_eff = bo + bv_full @ wo.T
    out = np.empty((L, N, C), np.float32)
    for b in range(N):
        out[:, b, :] = results[2 * b]["out"] + results[2 * b + 1]["out"] + bo_eff[None, :]
    return out


_NC_CACHE = {}


def _get_nc():
    if "nc" not in _NC_CACHE:
        _NC_CACHE["nc"] = build_nc()
    return _NC_CACHE["nc"]


def kernel(**inputs):
    from concourse import bass_utils
    nc = _get_nc()
    in_maps = marshal(inputs)
    res = bass_utils.run_bass_kernel_spmd(nc, in_maps, core_ids=list(range(8)))
    return gather(res.results, inputs)


# revision 3
# speedup vs baseline: 1.0054x; 1.0054x over previous
"""Trainium2 Bass kernel v3: sliding-window causal MHA with RoPE + ALiBi.

Sharding: 8 cores = 4 batches x 2 head-groups (8 heads each).

v3 = v2 structure + hi/lo-split fp8e4 DoubleRow projections:
  x @ w ~= xh@wh + xl@wh + xh@wl   (xh=fp8(x), xl=fp8(x-xh); w pre-scaled by
  32 host-side so wl avoids the fp8 subnormal floor; evac rescales by 1/32).
  Each term is a DoubleRow matmul contracting a PAIR of 128-c k-tiles
  (lhsT [128,2,M], rhs [128,2,N], both fp8) -> 24 matmuls per 512-wide psum
  instead of 16 bf16 matmuls, at 0.5 cyc/row: 0.75x cycles of bf16 overall.
  Validated on CPU: rel_err 3.0e-3 (better than all-bf16 3.39e-3).
Attention (scores, PV, ones-sums) stays bf16.  v-bias folded into host bo.
"""
import sys
sys.path.insert(0, '/opt/trn_rl_repo')
from contextlib import ExitStack

import numpy as np
import ml_dtypes
import concourse.bass as bass
import concourse.bacc as bacc
import concourse.mybir as mybir
import concourse.tile as tile

L, N, C, H, D, W = 1024, 4, 2048, 16, 128, 512
HPC = 8                       # heads per core
GD = HPC * D                  # 1024 head-dims per core
SCALE = 1.0 / float(np.sqrt(D))
WSCALE = 32.0                 # host pre-scale on weights (exact power of 2)
F32 = mybir.dt.float32
BF16 = mybir.dt.bfloat16
F8 = mybir.dt.float8e4
MMDT = BF16
DR = mybir.MatmulPerfMode.DoubleRow
AF = mybir.ActivationFunctionType
NT_C = C // 128               # 16 contraction tiles over embed dim
NT_HD = GD // 128             # 8 head tiles (1 head each, D=128)
NT_T = L // 128               # 8 token tiles
QG = 256                      # query-group width
NQG = L // QG                 # 4
MASK_W = 1408                 # master mask width: covers rel = dj - y + MASK_C0
MASK_C0 = 384
N_PREWARM = 105               # dummy matmuls to ramp PE clock during first DMAs


def jtiles(i0):
    return list(range(max(0, i0 - W), min(i0 + QG, L) - 128 + 1, 128))


def emit(tc, t):
    nc = tc.nc
    from concourse.alu_op_type import AluOpType

    # ---------------- prewarm + consts ----------------
    cpool = tc.alloc_tile_pool(name="const", bufs=1, side="left")
    warm = cpool.tile([128, 128], MMDT, tag="warm")
    nc.vector.memset(warm[:], 0.0)
    ones = cpool.tile([128, 128], MMDT, tag="ones")
    ones8 = cpool.tile([128, 2, 128], F8, tag="ones8")
    cos2 = cpool.tile([128, L], MMDT, tag="cos2")
    sin2 = cpool.tile([128, L], MMDT, tag="sin2")
    bq_s = cpool.tile([128, NT_HD], F32, tag="bq")
    bk_s = cpool.tile([128, NT_HD], F32, tag="bk")

    # long-lived left-stack tiles
    vp = tc.alloc_tile_pool(name="vp", bufs=1, side="left")
    vts = [vp.tile([128, GD], MMDT, tag=f"v{tt}", name=f"v{tt}") for tt in range(NT_T)]
    qkp = tc.alloc_tile_pool(name="qkp", bufs=1, side="left")
    qts = [qkp.tile([128, L], MMDT, tag=f"q{m}", name=f"q{m}") for m in range(NT_HD)]
    kts = [qkp.tile([128, L], MMDT, tag=f"k{m}", name=f"k{m}") for m in range(NT_HD)]
    # attention output, fp8 hi/lo packed [128, htile, 2, L]
    apool = tc.alloc_tile_pool(name="apool", bufs=1, side="left")
    aa = apool.tile([128, NT_HD, 2, L], F8, tag="aa")

    pool1 = tc.alloc_tile_pool(name="pool1", bufs=8, space="PSUM")

    def psum():
        return pool1.tile([128, 512], F32, tag="pp", name="pp")

    # prewarm: dummy matmuls on uninitialized tile (result never read)
    wps = psum()
    for i in range(N_PREWARM):
        nc.tensor.matmul(wps[:, 0:64], warm[:], warm[:, 0:64],
                         start=True, stop=True)

    # ---------------- phase A: v-proj ----------------
    xp = tc.alloc_tile_pool(name="xp", bufs=1, side="right")
    xx = xp.tile([128, NT_C, 2, L], F8, tag="xx")     # x hi/lo packed
    ws = tc.alloc_tile_pool(name="ws", bufs=4, side="right")
    rp = tc.alloc_tile_pool(name="rp", bufs=4, side="right")
    wvp = tc.alloc_tile_pool(name="wvp", bufs=1, side="right")
    wvv = wvp.tile([128, NT_C, 2, GD], F8, tag="wvv")  # wv*32 hi/lo packed
    for n in range(NT_C):
        (nc.sync if n % 2 == 0 else nc.scalar).dma_start(
            xx[:, n, :, :], t["x8"][n])
        (nc.scalar if n % 2 == 0 else nc.sync).dma_start(
            wvv[:, n, :, :], t["wv8"][n])
        if n == 1:
            nc.sync.dma_start(ones[:], t["ones"][:])
            nc.scalar.dma_start(ones8[:], t["ones8"][:])
    nc.sync.dma_start(cos2[:], t["cos2"][:])
    nc.scalar.dma_start(sin2[:], t["sin2"][:])
    nc.sync.dma_start(bq_s[:], t["bq"][:])
    nc.scalar.dma_start(bk_s[:], t["bk"][:])

    def hilo_mms(ps, lhs_t, rhs_t, kp, nkp, lslice, rslice):
        """3-term hi/lo DoubleRow matmuls for k-pair kp into psum ps.
        lhs_t/rhs_t are 4D packed tiles [128, nk, 2, M]."""
        first = (kp == 0)
        last = (kp == nkp - 1)
        terms = ((0, 0), (1, 0), (0, 1))   # (lhs hi/lo, rhs hi/lo)
        for ti, (la, rb) in enumerate(terms):
            nc.tensor.matmul(
                ps,
                lhs_t[:, 2 * kp:2 * kp + 2, la, lslice],
                rhs_t[:, 2 * kp:2 * kp + 2, rb, rslice],
                start=(first and ti == 0), stop=(last and ti == 2),
                perf_mode=DR)

    # wave 1: k-pair-outer over 8 banks (DMA-paced)
    pss = [psum() for _ in range(8)]
    for kp in range(8):
        for k in range(8):
            tt, i2 = k // 2, k % 2
            hilo_mms(pss[k][:], xx, wvv, kp, 8,
                     slice(tt * 128, (tt + 1) * 128),
                     slice(i2 * 512, (i2 + 1) * 512))
    for k in range(8):
        tt, i2 = k // 2, k % 2
        nc.scalar.activation(
            vts[tt][:, i2 * 512:(i2 + 1) * 512], pss[k][:],
            AF.Identity, scale=1.0 / WSCALE)
    # wave 2: kp-inner per bank (staggered completion)
    for k in range(8):
        tt, i2 = 4 + k // 2, k % 2
        ps = psum()
        for kp in range(8):
            hilo_mms(ps[:], xx, wvv, kp, 8,
                     slice(tt * 128, (tt + 1) * 128),
                     slice(i2 * 512, (i2 + 1) * 512))
        nc.scalar.activation(
            vts[tt][:, i2 * 512:(i2 + 1) * 512], ps[:],
            AF.Identity, scale=1.0 / WSCALE)

    wvp.release()

    # prefetch expb/wo into wvp's released space during phase B
    mpool = tc.alloc_tile_pool(name="mp", bufs=1, side="right")
    expbs = []
    for h in range(HPC):
        eb = mpool.tile([128, MASK_W], MMDT, tag=f"expb{h}", name=f"expb{h}")
        (nc.sync if h % 2 == 0 else nc.scalar).dma_start(eb[:], t["expb"][h])
        expbs.append(eb)
    wop = tc.alloc_tile_pool(name="wop", bufs=1, side="right")
    wov = wop.tile([128, NT_HD, 2, C], F8, tag="wov")  # wo*32 hi/lo packed
    for hh in range(NT_HD):
        (nc.scalar if hh % 2 == 0 else nc.sync).dma_start(
            wov[:, hh, :, :], t["wo8"][hh])

    # ---------------- phase B: q/k-proj + rope ----------------
    dq = 0
    for m in range(NT_HD):
        for wname, dst, bias_s in (("wq8", qts, bq_s), ("wk8", kts, bk_s)):
            wt = ws.tile([128, NT_C, 2, 128], F8, tag="wqk", name=f"w{wname}{m}")
            (nc.sync if dq % 2 == 0 else nc.scalar).dma_start(
                wt[:], t[wname][m])
            dq += 1
            for i2 in range(2):
                ps = psum()
                for kp in range(8):
                    # lhsT = weight tile [c,2,128 hd], rhs = x [c,2,512 t]
                    hilo_mms(ps[:], wt, xx, kp, 8,
                             slice(0, 128),
                             slice(i2 * 512, (i2 + 1) * 512))
                sl = dst[m][:, i2 * 512:(i2 + 1) * 512]
                csl = slice(i2 * 512, (i2 + 1) * 512)
                qw = rp.tile([128, 512], MMDT, tag="qw", name="qw")
                nc.scalar.activation(
                    qw[:], ps[:],
                    AF.Identity, bias=bias_s[:, m:m + 1], scale=1.0 / WSCALE)
                rot = rp.tile([128, 512], MMDT, tag="rot", name="rot")
                nc.vector.tensor_copy(rot[0:64, :], qw[64:128, :])
                nc.vector.tensor_copy(rot[64:128, :], qw[0:64, :])
                nc.vector.tensor_mul(qw[:], qw[:], cos2[:, csl])
                nc.vector.tensor_mul(rot[:], rot[:], sin2[:, csl])
                nc.vector.tensor_add(sl, qw[:], rot[:])

    pool1.release()

    # ---------------- phase C+D fused ----------------
    with tc.tile_pool(name="spp", bufs=4, space="PSUM") as spp, \
         tc.tile_pool(name="mpp", bufs=2, space="PSUM") as mpp, \
         tc.tile_pool(name="cw", bufs=6, side="right") as cw, \
         tc.tile_pool(name="og", bufs=3, side="right") as og:

        def banks_for(gi):
            """Pack j-tiles into score banks; edge tiles only cover the query
            half that is actually unmasked (low half for the window-start
            tile, high half for the diagonal tile)."""
            i0 = gi * QG
            js = jtiles(i0)
            tiles = []
            for jj in js:
                if gi >= 2 and jj == js[0]:
                    tiles.append((jj, 0, 128))        # low half only
                elif jj == js[-1]:
                    tiles.append((jj, 128, 128))      # high half only
                else:
                    tiles.append((jj, 0, QG))
            banks, cur, w = [], [], 0
            for t_ in tiles:
                if w + t_[2] > 512:
                    banks.append(cur)
                    cur, w = [], 0
                cur.append(t_)
                w += t_[2]
            if cur:
                banks.append(cur)
            return banks

        def emit_C(gi):
            i0 = gi * QG
            for h in range(HPC):
                ab = mpp.tile([128, 512], F32, tag="ab", name="ab", bufs=2)
                pts = []
                for bi, bank in enumerate(banks_for(gi)):
                    totw = sum(qw for _, _, qw in bank)
                    sb = spp.tile([128, 512], F32, tag="s", name="s_ps")
                    col = 0
                    for jj, qoff, qw in bank:
                        nc.tensor.matmul(
                            sb[:, col:col + qw],
                            kts[h][:, jj:jj + 128],
                            qts[h][:, i0 + qoff:i0 + qoff + qw],
                            start=True, stop=True)
                        col += qw
                    e = cw.tile([128, 512], MMDT, tag="e", name="e")
                    nc.scalar.activation(e[:, 0:totw], sb[:, 0:totw],
                                         AF.Exp, scale=SCALE)
                    col = 0
                    for ti, (jj, qoff, qw) in enumerate(bank):
                        pT = cw.tile([128, QG], MMDT, tag="pT", name="pT")
                        soff = MASK_C0 - (jj - (i0 + qoff))
                        eng = nc.vector if (bi + ti) % 2 == 0 else nc.gpsimd
                        eng.tensor_mul(pT[:, 0:qw], e[:, col:col + qw],
                                       expbs[h][:, soff:soff + qw])
                        col += qw
                        pts.append((jj, qoff, qw, pT))
                # full tiles first so the bank-zeroing start covers everything
                pts.sort(key=lambda p: p[2] != QG)
                # start=True zeroes the WHOLE psum bank (hw-verified), so only
                # the very first matmul starts; everything else accumulates.
                for idx, (jj, qoff, qw, pT) in enumerate(pts):
                    sp_ = (idx == len(pts) - 1)
                    nc.tensor.matmul(
                        ab[:, qoff:qoff + qw],
                        vts[jj // 128][:, h * 128:(h + 1) * 128],
                        pT[:, 0:qw], start=(idx == 0),
                        stop=sp_, skip_group_check=True)
                    nc.tensor.matmul(
                        ab[:, QG + qoff:QG + qoff + qw],
                        ones[:],
                        pT[:, 0:qw], start=False, stop=sp_,
                        skip_group_check=True)
                rec = cw.tile([128, QG], F32, tag="rec", name="rec", bufs=4)
                nc.vector.reciprocal(rec[:], ab[:, QG:2 * QG])
                aw = cw.tile([128, QG], F32, tag="aw", name="aw", bufs=4)
                nc.vector.scalar_tensor_tensor(
                    aw[:], ab[:, 0:QG], 1.0, rec[:],
                    op0=AluOpType.mult, op1=AluOpType.mult)
                # hi/lo fp8 split of attn row block
                asl = slice(i0, i0 + QG)
                nc.scalar.activation(aa[:, h, 0, asl], aw[:],
                                     AF.Identity, scale=1.0)
                nc.vector.tensor_sub(aa[:, h, 1, asl], aw[:], aa[:, h, 0, asl])

        def emit_D(gi):
            for tt in (2 * gi, 2 * gi + 1):
                tsl = slice(tt * 128, (tt + 1) * 128)
                for cc in range(4):
                    ps = mpp.tile([128, 512], F32, tag="po", name="psD", bufs=2)
                    for kp in range(NT_HD // 2):
                        hilo_mms(ps[:], aa, wov, kp, NT_HD // 2,
                                 tsl, slice(cc * 512, (cc + 1) * 512))
                    o = og.tile([128, 512], F32, tag="o", name="o")
                    if cc % 2 == 0:
                        nc.scalar.activation(o[:], ps[:], AF.Identity,
                                             scale=1.0 / WSCALE)
                    else:
                        nc.vector.tensor_scalar_mul(o[:], ps[:], 1.0 / WSCALE)
                    ((nc.sync if cc % 2 == 0 else nc.scalar)).dma_start(
                        t["out"][tsl, cc * 512:(cc + 1) * 512], o[:])

        # delay each group's out-proj by one group so its aa inputs are ready
        emit_C(0)
        for gi in range(1, NQG):
            emit_C(gi)
            emit_D(gi - 1)
        emit_D(NQG - 1)

    wop.release()
    mpool.release()
    rp.release()
    ws.release()
    xp.release()
    apool.release()
    qkp.release()
    vp.release()
    cpool.release()


def build_nc(enable_asserts=False, reps=1):
    nc = bacc.Bacc("TRN2", target_bir_lowering=False, debug=False,
                   enable_asserts=enable_asserts, num_devices=8)
    t = {}
    t["x8"] = nc.dram_tensor("x8", [NT_C, 128, 2, L], F8, kind="ExternalInput").ap()
    t["wv8"] = nc.dram_tensor("wv8", [NT_C, 128, 2, GD], F8, kind="ExternalInput").ap()
    t["wq8"] = nc.dram_tensor("wq8", [NT_HD, 128, NT_C, 2, 128], F8, kind="ExternalInput").ap()
    t["wk8"] = nc.dram_tensor("wk8", [NT_HD, 128, NT_C, 2, 128], F8, kind="ExternalInput").ap()
    t["wo8"] = nc.dram_tensor("wo8", [NT_HD, 128, 2, C], F8, kind="ExternalInput").ap()
    t["cos2"] = nc.dram_tensor("cos2", [128, L], MMDT, kind="ExternalInput").ap()
    t["sin2"] = nc.dram_tensor("sin2", [128, L], MMDT, kind="ExternalInput").ap()
    t["bq"] = nc.dram_tensor("bq", [128, NT_HD], F32, kind="ExternalInput").ap()
    t["bk"] = nc.dram_tensor("bk", [128, NT_HD], F32, kind="ExternalInput").ap()
    t["expb"] = nc.dram_tensor("expb", [HPC, 128, MASK_W], MMDT, kind="ExternalInput").ap()
    t["ones"] = nc.dram_tensor("ones", [128, 128], MMDT, kind="ExternalInput").ap()
    t["ones8"] = nc.dram_tensor("ones8", [128, 2, 128], F8, kind="ExternalInput").ap()
    t["out"] = nc.dram_tensor("out", [L, C], F32, kind="ExternalOutput").ap()
    with tile.TileContext(nc) as tc:
        for _ in range(reps):
            emit(tc, t)
    nc.compile()
    return nc


F8NP = ml_dtypes.float8_e4m3fn


def split8(a):
    hi = np.asarray(a, np.float32).astype(F8NP)
    lo = (np.asarray(a, np.float32) - hi.astype(np.float32)).astype(F8NP)
    return hi, lo


def marshal(inputs):
    x = np.asarray(inputs["x"], np.float32)
    wq = np.asarray(inputs["wq"], np.float32)
    wkv = np.asarray(inputs["wkv"], np.float32)
    wo = np.asarray(inputs["wo"], np.float32)
    bq = np.asarray(inputs["bq"], np.float32)
    bkv = np.asarray(inputs["bkv"], np.float32)
    alibi = np.asarray(inputs["alibi_slopes"], np.float32)
    wk_full, wv_full = wkv[:C], wkv[C:]
    bk_full = bkv[:C]

    perm = np.concatenate([np.arange(0, D, 2), np.arange(1, D, 2)])
    head_perm = np.concatenate([h * D + perm for h in range(H)])
    wq_p, wk_p = wq[head_perm], wk_full[head_perm]
    bq_p, bk_p = bq[head_perm], bk_full[head_perm]

    t_abs = np.arange(W, W + L, dtype=np.float64)
    inv = 1.0 / (10000.0 ** (np.arange(0, D, 2, dtype=np.float64) / D))
    fr = np.outer(t_abs, inv)
    cosT = np.cos(fr).T.astype(np.float32)
    sinT = np.sin(fr).T.astype(np.float32)
    cos2 = np.ascontiguousarray(np.concatenate([cosT, cosT], 0))
    sin2 = np.ascontiguousarray(np.concatenate([-sinT, sinT], 0))

    dj = np.arange(128)[:, None]
    y = np.arange(MASK_W)[None, :]
    rel = (dj - y + MASK_C0).astype(np.float64)
    win = (rel <= 0) & (rel >= -W)

    bf = ml_dtypes.bfloat16
    in_maps = []
    for core in range(8):
        b, g = divmod(core, 2)
        gs = slice(g * GD, (g + 1) * GD)
        xb = x[:, b, :]
        # x8: [NT_C, 128, 2, L] hi/lo packed
        xT_m = np.ascontiguousarray(xb.T).reshape(NT_C, 128, L)
        xh, xl = split8(xT_m)
        x8 = np.ascontiguousarray(np.stack([xh, xl], axis=2))
        # wv8: [NT_C, 128, 2, GD], scaled by 32
        wv_m = np.ascontiguousarray(wv_full[gs].T).reshape(NT_C, 128, GD)
        wvh, wvl = split8(wv_m * WSCALE)
        wv8 = np.ascontiguousarray(np.stack([wvh, wvl], axis=2))
        # wq8/wk8: [NT_HD, 128, NT_C, 2, 128], scaled by 32
        wq_m = wq_p[gs].reshape(NT_HD, 128, NT_C, 128).transpose(0, 3, 2, 1)
        wk_m = wk_p[gs].reshape(NT_HD, 128, NT_C, 128).transpose(0, 3, 2, 1)
        wqh, wql = split8(wq_m * WSCALE)
        wkh, wkl = split8(wk_m * WSCALE)
        wq8 = np.ascontiguousarray(np.stack([wqh, wql], axis=3))
        wk8 = np.ascontiguousarray(np.stack([wkh, wkl], axis=3))
        # wo8: [NT_HD, 128, 2, C] (hd-inner on partition), scaled by 32
        wo_m = np.ascontiguousarray(wo[:, gs].T).reshape(NT_HD, 128, C)
        woh, wol = split8(wo_m * WSCALE)
        wo8 = np.ascontiguousarray(np.stack([woh, wol], axis=2))
        bq_m = np.ascontiguousarray(bq_p[gs].reshape(NT_HD, 128).T)
        bk_m = np.ascontiguousarray(bk_p[gs].reshape(NT_HD, 128).T)
        expb = np.zeros((HPC, 128, MASK_W), np.float32)
        for hh in range(HPC):
            s = float(alibi[g * HPC + hh])
            expb[hh] = np.where(win, np.exp(s * rel), 0.0).astype(np.float32)
        in_maps.append(dict(
            x8=x8, wv8=wv8, wq8=wq8, wk8=wk8, wo8=wo8,
            cos2=cos2.astype(bf), sin2=sin2.astype(bf),
            bq=bq_m, bk=bk_m, expb=expb.astype(bf),
            ones=np.ones((128, 128), bf),
            ones8=np.ones((128, 2, 128), F8NP)))
    return in_maps


def gather(results, inputs):
    wo = np.asarray(inputs["wo"], np.float32)
    bo = np.asarray(inputs["bo"], np.float32)
    bv_full = np.asarray(inputs["bkv"], np.float32)[C:]
    bo_eff = bo + bv_full @ wo.T
    out = np.empty((L, N, C), np.float32)
    for b in range(N):
        out[:, b, :] = results[2 * b]["out"] + results[2 * b + 1]["out"] + bo_eff[None, :]
    return out


_NC_CACHE = {}


def _get_nc():
    if "nc" not in _NC_CACHE:
        _NC_CACHE["nc"] = build_nc()
    return _NC_CACHE["nc"]


def kernel(**inputs):
    from concourse import bass_utils
    nc = _get_nc()
    in_maps = marshal(inputs)
    res = bass_utils.run_bass_kernel_spmd(nc, in_maps, core_ids=list(range(8)))
    return gather(res.results, inputs)


# revision 4
# speedup vs baseline: 1.0158x; 1.0104x over previous
"""Trainium2 Bass kernel v3: sliding-window causal MHA with RoPE + ALiBi.

Sharding: 8 cores = 4 batches x 2 head-groups (8 heads each).

v3 = v2 structure + hi/lo-split fp8e4 DoubleRow projections:
  x @ w ~= xh@wh + xl@wh + xh@wl   (xh=fp8(x), xl=fp8(x-xh); w pre-scaled by
  32 host-side so wl avoids the fp8 subnormal floor; evac rescales by 1/32).
  Each term is a DoubleRow matmul contracting a PAIR of 128-c k-tiles
  (lhsT [128,2,M], rhs [128,2,N], both fp8) -> 24 matmuls per 512-wide psum
  instead of 16 bf16 matmuls, at 0.5 cyc/row: 0.75x cycles of bf16 overall.
  Validated on CPU: rel_err 3.0e-3 (better than all-bf16 3.39e-3).
Attention (scores, PV, ones-sums) stays bf16.  v-bias folded into host bo.
"""
import sys
sys.path.insert(0, '/opt/trn_rl_repo')
from contextlib import ExitStack

import numpy as np
import ml_dtypes
import concourse.bass as bass
import concourse.bacc as bacc
import concourse.mybir as mybir
import concourse.tile as tile

L, N, C, H, D, W = 1024, 4, 2048, 16, 128, 512
HPC = 8                       # heads per core
GD = HPC * D                  # 1024 head-dims per core
SCALE = 1.0 / float(np.sqrt(D))
WSCALE = 32.0                 # host pre-scale on weights (exact power of 2)
F32 = mybir.dt.float32
BF16 = mybir.dt.bfloat16
F8 = mybir.dt.float8e4
MMDT = BF16
DR = mybir.MatmulPerfMode.DoubleRow
AF = mybir.ActivationFunctionType
NT_C = C // 128               # 16 contraction tiles over embed dim
NT_HD = GD // 128             # 8 head tiles (1 head each, D=128)
NT_T = L // 128               # 8 token tiles
QG = 256                      # query-group width
NQG = L // QG                 # 4
MASK_W = 1408                 # master mask width: covers rel = dj - y + MASK_C0
MASK_C0 = 384
N_PREWARM = 105               # dummy matmuls to ramp PE clock during first DMAs


def jtiles(i0):
    return list(range(max(0, i0 - W), min(i0 + QG, L) - 128 + 1, 128))


def emit(tc, t):
    nc = tc.nc
    from concourse.alu_op_type import AluOpType

    # ---------------- prewarm + consts ----------------
    cpool = tc.alloc_tile_pool(name="const", bufs=1, side="left")
    warm = cpool.tile([128, 128], MMDT, tag="warm")
    nc.vector.memset(warm[:], 0.0)
    ones = cpool.tile([128, 128], MMDT, tag="ones")
    ones8 = cpool.tile([128, 2, 128], F8, tag="ones8")
    cos2 = cpool.tile([128, L], MMDT, tag="cos2")
    sin2 = cpool.tile([128, L], MMDT, tag="sin2")
    bq_s = cpool.tile([128, NT_HD], F32, tag="bq")
    bk_s = cpool.tile([128, NT_HD], F32, tag="bk")

    # long-lived left-stack tiles
    vp = tc.alloc_tile_pool(name="vp", bufs=1, side="left")
    vts = [vp.tile([128, GD], MMDT, tag=f"v{tt}", name=f"v{tt}") for tt in range(NT_T)]
    qkp = tc.alloc_tile_pool(name="qkp", bufs=1, side="left")
    qts = [qkp.tile([128, L], MMDT, tag=f"q{m}", name=f"q{m}") for m in range(NT_HD)]
    kts = [qkp.tile([128, L], MMDT, tag=f"k{m}", name=f"k{m}") for m in range(NT_HD)]
    # attention output, fp8 hi/lo packed [128, htile, 2, L]
    apool = tc.alloc_tile_pool(name="apool", bufs=1, side="left")
    aa = apool.tile([128, NT_HD, 2, L], F8, tag="aa")

    pool1 = tc.alloc_tile_pool(name="pool1", bufs=8, space="PSUM")

    def psum():
        return pool1.tile([128, 512], F32, tag="pp", name="pp")

    # prewarm: dummy matmuls on uninitialized tile (result never read)
    wps = psum()
    for i in range(N_PREWARM):
        nc.tensor.matmul(wps[:, 0:64], warm[:], warm[:, 0:64],
                         start=True, stop=True)

    # ---------------- phase A: v-proj ----------------
    xp = tc.alloc_tile_pool(name="xp", bufs=1, side="right")
    xx = xp.tile([128, NT_C, 2, L], F8, tag="xx")     # x hi/lo packed
    ws = tc.alloc_tile_pool(name="ws", bufs=4, side="right")
    rp = tc.alloc_tile_pool(name="rp", bufs=4, side="right")
    wvp = tc.alloc_tile_pool(name="wvp", bufs=1, side="right")
    wvv = wvp.tile([128, NT_C, 2, GD], F8, tag="wvv")  # wv*32 hi/lo packed
    for n in range(NT_C):
        (nc.sync if n % 2 == 0 else nc.scalar).dma_start(
            xx[:, n, :, :], t["x8"][n])
        (nc.scalar if n % 2 == 0 else nc.sync).dma_start(
            wvv[:, n, :, :], t["wv8"][n])
        if n == 1:
            nc.sync.dma_start(ones[:], t["ones"][:])
            nc.scalar.dma_start(ones8[:], t["ones8"][:])
    nc.sync.dma_start(cos2[:], t["cos2"][:])
    nc.scalar.dma_start(sin2[:], t["sin2"][:])
    nc.sync.dma_start(bq_s[:], t["bq"][:])
    nc.scalar.dma_start(bk_s[:], t["bk"][:])

    def hilo_mms(ps, lhs_t, rhs_t, kp, nkp, lslice, rslice):
        """3-term hi/lo DoubleRow matmuls for k-pair kp into psum ps.
        lhs_t/rhs_t are 4D packed tiles [128, nk, 2, M]."""
        first = (kp == 0)
        last = (kp == nkp - 1)
        terms = ((0, 0), (1, 0), (0, 1))   # (lhs hi/lo, rhs hi/lo)
        for ti, (la, rb) in enumerate(terms):
            nc.tensor.matmul(
                ps,
                lhs_t[:, 2 * kp:2 * kp + 2, la, lslice],
                rhs_t[:, 2 * kp:2 * kp + 2, rb, rslice],
                start=(first and ti == 0), stop=(last and ti == 2),
                perf_mode=DR)

    # wave 1: k-pair-outer over 8 banks (DMA-paced)
    pss = [psum() for _ in range(8)]
    for kp in range(8):
        for k in range(8):
            tt, i2 = k // 2, k % 2
            hilo_mms(pss[k][:], xx, wvv, kp, 8,
                     slice(tt * 128, (tt + 1) * 128),
                     slice(i2 * 512, (i2 + 1) * 512))
    for k in range(8):
        tt, i2 = k // 2, k % 2
        nc.scalar.activation(
            vts[tt][:, i2 * 512:(i2 + 1) * 512], pss[k][:],
            AF.Identity, scale=1.0 / WSCALE)
    # wave 2: kp-inner per bank (staggered completion)
    for k in range(8):
        tt, i2 = 4 + k // 2, k % 2
        ps = psum()
        for kp in range(8):
            hilo_mms(ps[:], xx, wvv, kp, 8,
                     slice(tt * 128, (tt + 1) * 128),
                     slice(i2 * 512, (i2 + 1) * 512))
        nc.scalar.activation(
            vts[tt][:, i2 * 512:(i2 + 1) * 512], ps[:],
            AF.Identity, scale=1.0 / WSCALE)

    wvp.release()

    # prefetch expb/wo into wvp's released space during phase B
    mpool = tc.alloc_tile_pool(name="mp", bufs=1, side="right")
    expbs = []
    for h in range(HPC):
        eb = mpool.tile([128, MASK_W], MMDT, tag=f"expb{h}", name=f"expb{h}")
        (nc.sync if h % 2 == 0 else nc.scalar).dma_start(eb[:], t["expb"][h])
        expbs.append(eb)
    wop = tc.alloc_tile_pool(name="wop", bufs=1, side="right")
    wov = wop.tile([128, NT_HD, 2, C], F8, tag="wov")  # wo*32 hi/lo packed
    for hh in range(NT_HD):
        (nc.scalar if hh % 2 == 0 else nc.sync).dma_start(
            wov[:, hh, :, :], t["wo8"][hh])

    # ---------------- phase B: q/k-proj + rope ----------------
    dq = 0
    for m in range(NT_HD):
        for wname, dst, bias_s in (("wq8", qts, bq_s), ("wk8", kts, bk_s)):
            wt = ws.tile([128, NT_C, 2, 128], F8, tag="wqk", name=f"w{wname}{m}")
            (nc.sync if dq % 2 == 0 else nc.scalar).dma_start(
                wt[:], t[wname][m])
            dq += 1
            for i2 in range(2):
                ps = psum()
                for kp in range(8):
                    # lhsT = weight tile [c,2,128 hd], rhs = x [c,2,512 t]
                    hilo_mms(ps[:], wt, xx, kp, 8,
                             slice(0, 128),
                             slice(i2 * 512, (i2 + 1) * 512))
                sl = dst[m][:, i2 * 512:(i2 + 1) * 512]
                csl = slice(i2 * 512, (i2 + 1) * 512)
                qw = rp.tile([128, 512], MMDT, tag="qw", name="qw")
                nc.scalar.activation(
                    qw[:], ps[:],
                    AF.Identity, bias=bias_s[:, m:m + 1], scale=1.0 / WSCALE)
                rot = rp.tile([128, 512], MMDT, tag="rot", name="rot")
                nc.vector.tensor_copy(rot[0:64, :], qw[64:128, :])
                nc.vector.tensor_copy(rot[64:128, :], qw[0:64, :])
                nc.vector.tensor_mul(qw[:], qw[:], cos2[:, csl])
                nc.vector.tensor_mul(rot[:], rot[:], sin2[:, csl])
                nc.vector.tensor_add(sl, qw[:], rot[:])

    pool1.release()

    # ---------------- phase C+D fused ----------------
    with tc.tile_pool(name="spp", bufs=4, space="PSUM") as spp, \
         tc.tile_pool(name="mpp", bufs=2, space="PSUM") as mpp, \
         tc.tile_pool(name="cw", bufs=6, side="right") as cw, \
         tc.tile_pool(name="og", bufs=4, side="right") as og:

        def banks_for(gi):
            """Pack j-tiles into score banks; edge tiles only cover the query
            half that is actually unmasked (low half for the window-start
            tile, high half for the diagonal tile)."""
            i0 = gi * QG
            js = jtiles(i0)
            tiles = []
            for jj in js:
                if gi >= 2 and jj == js[0]:
                    tiles.append((jj, 0, 128))        # low half only
                elif jj == js[-1]:
                    tiles.append((jj, 128, 128))      # high half only
                else:
                    tiles.append((jj, 0, QG))
            banks, cur, w = [], [], 0
            for t_ in tiles:
                if w + t_[2] > 512:
                    banks.append(cur)
                    cur, w = [], 0
                cur.append(t_)
                w += t_[2]
            if cur:
                banks.append(cur)
            return banks

        def emit_C(gi):
            i0 = gi * QG
            for h in range(HPC):
                ab = mpp.tile([128, 512], F32, tag="ab", name="ab", bufs=2)
                pts = []
                for bi, bank in enumerate(banks_for(gi)):
                    totw = sum(qw for _, _, qw in bank)
                    sb = spp.tile([128, 512], F32, tag="s", name="s_ps")
                    col = 0
                    for jj, qoff, qw in bank:
                        nc.tensor.matmul(
                            sb[:, col:col + qw],
                            kts[h][:, jj:jj + 128],
                            qts[h][:, i0 + qoff:i0 + qoff + qw],
                            start=True, stop=True)
                        col += qw
                    e = cw.tile([128, 512], MMDT, tag="e", name="e", bufs=8)
                    nc.scalar.activation(e[:, 0:totw], sb[:, 0:totw],
                                         AF.Exp, scale=SCALE)
                    col = 0
                    for ti, (jj, qoff, qw) in enumerate(bank):
                        pT = cw.tile([128, QG], MMDT, tag="pT", name="pT")
                        soff = MASK_C0 - (jj - (i0 + qoff))
                        eng = nc.vector if (bi + ti) % 2 == 0 else nc.gpsimd
                        eng.tensor_mul(pT[:, 0:qw], e[:, col:col + qw],
                                       expbs[h][:, soff:soff + qw])
                        col += qw
                        pts.append((jj, qoff, qw, pT))
                # full tiles first so the bank-zeroing start covers everything
                pts.sort(key=lambda p: p[2] != QG)
                # start=True zeroes the WHOLE psum bank (hw-verified), so only
                # the very first matmul starts; everything else accumulates.
                for idx, (jj, qoff, qw, pT) in enumerate(pts):
                    sp_ = (idx == len(pts) - 1)
                    nc.tensor.matmul(
                        ab[:, qoff:qoff + qw],
                        vts[jj // 128][:, h * 128:(h + 1) * 128],
                        pT[:, 0:qw], start=(idx == 0),
                        stop=sp_, skip_group_check=True)
                    nc.tensor.matmul(
                        ab[:, QG + qoff:QG + qoff + qw],
                        ones[:],
                        pT[:, 0:qw], start=False, stop=sp_,
                        skip_group_check=True)
                rec = cw.tile([128, QG], F32, tag="rec", name="rec", bufs=4)
                nc.vector.reciprocal(rec[:], ab[:, QG:2 * QG])
                aw = cw.tile([128, QG], F32, tag="aw", name="aw", bufs=4)
                nc.vector.scalar_tensor_tensor(
                    aw[:], ab[:, 0:QG], 1.0, rec[:],
                    op0=AluOpType.mult, op1=AluOpType.mult)
                # hi/lo fp8 split of attn row block
                asl = slice(i0, i0 + QG)
                nc.scalar.activation(aa[:, h, 0, asl], aw[:],
                                     AF.Identity, scale=1.0)
                nc.vector.tensor_sub(aa[:, h, 1, asl], aw[:], aa[:, h, 0, asl])

        def emit_D(gi, fine=False):
            for tt in (2 * gi, 2 * gi + 1):
                tsl = slice(tt * 128, (tt + 1) * 128)
                # final token tile: split the last chunk so the tail
                # evac+DMA chain is short
                chunks = [(0, 512), (512, 512), (1024, 512), (1536, 512)]
                for cc, (c0, cw_) in enumerate(chunks):
                    ps = mpp.tile([128, 512], F32, tag="po", name="psD", bufs=2)
                    for kp in range(NT_HD // 2):
                        hilo_mms(ps[:, 0:cw_], aa, wov, kp, NT_HD // 2,
                                 tsl, slice(c0, c0 + cw_))
                    o = og.tile([128, 512], F32, tag="o", name="o")
                    last = fine and tt == 2 * gi + 1 and cc == 3
                    if last:
                        # split final evac+store across engines/queues to
                        # shorten the end-of-kernel chain
                        nc.scalar.activation(o[:, 0:256], ps[:, 0:256],
                                             AF.Identity, scale=1.0 / WSCALE)
                        nc.vector.tensor_scalar_mul(o[:, 256:512],
                                                    ps[:, 256:512],
                                                    1.0 / WSCALE)
                        nc.sync.dma_start(t["out"][tsl, c0:c0 + 256],
                                          o[:, 0:256])
                        nc.scalar.dma_start(t["out"][tsl, c0 + 256:c0 + 512],
                                            o[:, 256:512])
                        continue
                    if cc % 2 == 0:
                        nc.scalar.activation(o[:, 0:cw_], ps[:, 0:cw_],
                                             AF.Identity, scale=1.0 / WSCALE)
                    else:
                        nc.vector.tensor_scalar_mul(o[:, 0:cw_], ps[:, 0:cw_],
                                                    1.0 / WSCALE)
                    ((nc.sync if cc % 2 == 0 else nc.scalar)).dma_start(
                        t["out"][tsl, c0:c0 + cw_], o[:, 0:cw_])

        # delay each group's out-proj by one group so its aa inputs are ready
        emit_C(0)
        for gi in range(1, NQG):
            emit_C(gi)
            emit_D(gi - 1)
        emit_D(NQG - 1, fine=False)

    wop.release()
    mpool.release()
    rp.release()
    ws.release()
    xp.release()
    apool.release()
    qkp.release()
    vp.release()
    cpool.release()


def build_nc(enable_asserts=False, reps=1):
    nc = bacc.Bacc("TRN2", target_bir_lowering=False, debug=False,
                   enable_asserts=enable_asserts, num_devices=8)
    t = {}
    t["x8"] = nc.dram_tensor("x8", [NT_C, 128, 2, L], F8, kind="ExternalInput").ap()
    t["wv8"] = nc.dram_tensor("wv8", [NT_C, 128, 2, GD], F8, kind="ExternalInput").ap()
    t["wq8"] = nc.dram_tensor("wq8", [NT_HD, 128, NT_C, 2, 128], F8, kind="ExternalInput").ap()
    t["wk8"] = nc.dram_tensor("wk8", [NT_HD, 128, NT_C, 2, 128], F8, kind="ExternalInput").ap()
    t["wo8"] = nc.dram_tensor("wo8", [NT_HD, 128, 2, C], F8, kind="ExternalInput").ap()
    t["cos2"] = nc.dram_tensor("cos2", [128, L], MMDT, kind="ExternalInput").ap()
    t["sin2"] = nc.dram_tensor("sin2", [128, L], MMDT, kind="ExternalInput").ap()
    t["bq"] = nc.dram_tensor("bq", [128, NT_HD], F32, kind="ExternalInput").ap()
    t["bk"] = nc.dram_tensor("bk", [128, NT_HD], F32, kind="ExternalInput").ap()
    t["expb"] = nc.dram_tensor("expb", [HPC, 128, MASK_W], MMDT, kind="ExternalInput").ap()
    t["ones"] = nc.dram_tensor("ones", [128, 128], MMDT, kind="ExternalInput").ap()
    t["ones8"] = nc.dram_tensor("ones8", [128, 2, 128], F8, kind="ExternalInput").ap()
    t["out"] = nc.dram_tensor("out", [L, C], F32, kind="ExternalOutput").ap()
    with tile.TileContext(nc) as tc:
        for _ in range(reps):
            emit(tc, t)
    nc.compile()
    return nc


F8NP = ml_dtypes.float8_e4m3fn


def split8(a):
    hi = np.asarray(a, np.float32).astype(F8NP)
    lo = (np.asarray(a, np.float32) - hi.astype(np.float32)).astype(F8NP)
    return hi, lo


def marshal(inputs):
    x = np.asarray(inputs["x"], np.float32)
    wq = np.asarray(inputs["wq"], np.float32)
    wkv = np.asarray(inputs["wkv"], np.float32)
    wo = np.asarray(inputs["wo"], np.float32)
    bq = np.asarray(inputs["bq"], np.float32)
    bkv = np.asarray(inputs["bkv"], np.float32)
    alibi = np.asarray(inputs["alibi_slopes"], np.float32)
    wk_full, wv_full = wkv[:C], wkv[C:]
    bk_full = bkv[:C]

    perm = np.concatenate([np.arange(0, D, 2), np.arange(1, D, 2)])
    head_perm = np.concatenate([h * D + perm for h in range(H)])
    wq_p, wk_p = wq[head_perm], wk_full[head_perm]
    bq_p, bk_p = bq[head_perm], bk_full[head_perm]

    t_abs = np.arange(W, W + L, dtype=np.float64)
    inv = 1.0 / (10000.0 ** (np.arange(0, D, 2, dtype=np.float64) / D))
    fr = np.outer(t_abs, inv)
    cosT = np.cos(fr).T.astype(np.float32)
    sinT = np.sin(fr).T.astype(np.float32)
    cos2 = np.ascontiguousarray(np.concatenate([cosT, cosT], 0))
    sin2 = np.ascontiguousarray(np.concatenate([-sinT, sinT], 0))

    dj = np.arange(128)[:, None]
    y = np.arange(MASK_W)[None, :]
    rel = (dj - y + MASK_C0).astype(np.float64)
    win = (rel <= 0) & (rel >= -W)

    bf = ml_dtypes.bfloat16
    in_maps = []
    for core in range(8):
        b, g = divmod(core, 2)
        gs = slice(g * GD, (g + 1) * GD)
        xb = x[:, b, :]
        # x8: [NT_C, 128, 2, L] hi/lo packed
        xT_m = np.ascontiguousarray(xb.T).reshape(NT_C, 128, L)
        xh, xl = split8(xT_m)
        x8 = np.ascontiguousarray(np.stack([xh, xl], axis=2))
        # wv8: [NT_C, 128, 2, GD], scaled by 32
        wv_m = np.ascontiguousarray(wv_full[gs].T).reshape(NT_C, 128, GD)
        wvh, wvl = split8(wv_m * WSCALE)
        wv8 = np.ascontiguousarray(np.stack([wvh, wvl], axis=2))
        # wq8/wk8: [NT_HD, 128, NT_C, 2, 128], scaled by 32
        wq_m = wq_p[gs].reshape(NT_HD, 128, NT_C, 128).transpose(0, 3, 2, 1)
        wk_m = wk_p[gs].reshape(NT_HD, 128, NT_C, 128).transpose(0, 3, 2, 1)
        wqh, wql = split8(wq_m * WSCALE)
        wkh, wkl = split8(wk_m * WSCALE)
        wq8 = np.ascontiguousarray(np.stack([wqh, wql], axis=3))
        wk8 = np.ascontiguousarray(np.stack([wkh, wkl], axis=3))
        # wo8: [NT_HD, 128, 2, C] (hd-inner on partition), scaled by 32
        wo_m = np.ascontiguousarray(wo[:, gs].T).reshape(NT_HD, 128, C)
        woh, wol = split8(wo_m * WSCALE)
        wo8 = np.ascontiguousarray(np.stack([woh, wol], axis=2))
        bq_m = np.ascontiguousarray(bq_p[gs].reshape(NT_HD, 128).T)
        bk_m = np.ascontiguousarray(bk_p[gs].reshape(NT_HD, 128).T)
        expb = np.zeros((HPC, 128, MASK_W), np.float32)
        for hh in range(HPC):
            s = float(alibi[g * HPC + hh])
            expb[hh] = np.where(win, np.exp(s * rel), 0.0).astype(np.float32)
        in_maps.append(dict(
            x8=x8, wv8=wv8, wq8=wq8, wk8=wk8, wo8=wo8,
            cos2=cos2.astype(bf), sin2=sin2.astype(bf),
            bq=bq_m, bk=bk_m, expb=expb.astype(bf),
            ones=np.ones((128, 128), bf),
            ones8=np.ones((128, 2, 128), F8NP)))
    return in_maps


def gather(results, inputs):
    wo = np.asarray(inputs["wo"], np.float32)
    bo = np.asarray(inputs["bo"], np.float32)
    bv_full = np.asarray(inputs["bkv"], np.float32)[C:]
    bo_eff = bo + bv_full @ wo.T
    out = np.empty((L, N, C), np.float32)
    for b in range(N):
        out[:, b, :] = results[2 * b]["out"] + results[2 * b + 1]["out"] + bo_eff[None, :]
    return out


_NC_CACHE = {}


def _get_nc():
    if "nc" not in _NC_CACHE:
        _NC_CACHE["nc"] = build_nc()
    return _NC_CACHE["nc"]


def kernel(**inputs):
    from concourse import bass_utils
    nc = _get_nc()
    in_maps = marshal(inputs)
    res = bass_utils.run_bass_kernel_spmd(nc, in_maps, core_ids=list(range(8)))
    return gather(res.results, inputs)
